# revision 8
# baseline (speedup 1.0000x reference)
"""Trainium2 Bass kernel for the nn_BertForOrdering pointer-network loss.

Row-interleaved valid-region kernel.

Sharding: core c handles rows t ≡ c (mod 8) of EVERY batch element, but
only t < ceil(L_b/8)*8 and columns j < L_b (the valid region — masked
entries of the score matrix never affect the loss beyond their exact -1e9
count, which the host reproduces).  All 8 cores run the same program
(uniform shapes; per-core data differs only in DRAM contents).  Column
softmax is computed as per-core partials (max, sumexp) and combined on
the host; row softmax rows live entirely on one core.
"""

import ml_dtypes
import numpy as np

import bass_rust
import concourse.bass as bass
import concourse.tile as tile
from concourse import mybir
from concourse.bass_utils import run_bass_kernel_spmd
from concourse.vector_clock import ScopedClock

class SafeTileContext(tile.TileContext):
    """Splits the tail-drain's sem waits into 1-wait carrier instructions:
    the walrus build in this container caps sync-wait commands per
    instruction at 1."""

    MAXW = 1

    def _drain_and_barrier(self, tick_clock, wait_clock):
        nc = self.nc
        drain_inst = nc.sync.drain()
        wait_clock.add_sem_waits(
            drain_inst.ins, ScopedClock({None: tick_clock.global_clock})
        )
        si = drain_inst.ins.sync_info
        if si is not None and len(si.on_wait) > self.MAXW:
            waits = list(si.on_wait)
            drain_inst.ins.sync_info = bass_rust.SyncInfo(
                on_wait=waits[: self.MAXW], on_update=list(si.on_update)
            )
            for i in range(self.MAXW, len(waits), self.MAXW):
                extra = nc.sync.drain()
                extra.ins.sync_info = bass_rust.SyncInfo(
                    on_wait=waits[i : i + self.MAXW], on_update=[]
                )
        nc.all_engine_barrier()
        assert self.sems is not None
        popped = nc._tile_sem_poison_stack.pop()
        assert popped is self._sem_poison
        nc.clear_and_free_semaphores(list(self.sems.allocated().values()))
        nc.all_engine_barrier()


def _split_waits(nc, maxw=1):
    """Move excess sync waits onto NOP carriers inserted immediately before
    the instruction in block order (same engine stream -> same semantics)."""

    def carrier(engine):
        bi = nc.engines[engine].nop(nofuse=True)
        ins = bi.ins
        for bb in nc.main_func.blocks:
            lst = bb.instructions
            if lst and lst[-1] is ins:
                lst.pop()
                break
        return ins

    for bb in nc.main_func.blocks:
        lst = bb.instructions
        new = []
        for ins in lst:
            si = ins.sync_info
            if si is not None and len(si.on_wait) > maxw:
                waits = list(si.on_wait)
                keep = waits[-maxw:]
                extra = waits[:-maxw]
                for k in range(0, len(extra), maxw):
                    nop = carrier(ins.engine)
                    nop.sync_info = bass_rust.SyncInfo(
                        on_wait=extra[k : k + maxw], on_update=[]
                    )
                    new.append(nop)
                ins.sync_info = bass_rust.SyncInfo(
                    on_wait=keep, on_update=list(si.on_update)
                )
            new.append(ins)
        lst[:] = new



B, N, H = 16, 128, 768
NCORES = 8
HC = H // 128
NEG = np.float32(-1e9)
F32 = mybir.dt.float32
BF16 = mybir.dt.bfloat16


def _plan(Ls):
    """Static schedule derived from tgt_len values (same on every core)."""
    Ls = [int(x) for x in Ls]
    nrows = [-(-L // 8) for L in Ls]
    Lp = [L + (L & 1) for L in Ls]   # even widths: keeps bf16 DVE in 2x mode
    ro = np.concatenate([[0], np.cumsum(nrows)]).astype(int)  # row offsets
    ko = np.concatenate([[0], np.cumsum(Lp)]).astype(int)     # kT col offsets
    S = int(ro[-1])
    SK = int(ko[-1])
    NRT = -(-S // 128)
    # balance: move trailing rows (t-units) of large-L batches from the
    # DVE-add path to the ACT bias-tanh path until engine times equalize
    dve = 13000.0 + sum(
        6 * (93 + Lp[b] / 2) / 0.96 for b in range(B) for _ in range(nrows[b])
    )
    act = (
        sum(6 * nrows[b] * Lp[b] / 1.2 for b in range(B))
        + 16 * 352 / 1.2
        + 14000.0  # exp + misc + psum copies
    )
    na = [0] * B
    units = sorted(
        [(Lp[b], b) for b in range(B) for _ in range(nrows[b])], reverse=True
    )
    for L, b in units:
        save = 6 * (93 + L / 2) / 0.96
        cost = 6 * 352 / 1.2
        if dve > act + save:
            na[b] += 1
            dve -= save
            act += cost
        else:
            break
    nd = [nrows[b] - na[b] for b in range(B)]
    return dict(
        Ls=Ls, Lp=Lp, nrows=nrows, ro=ro, ko=ko, S=S, SK=SK, NRT=NRT, nd=nd, na=na
    )


def _build_program_v2(plan, ebufs=3):
    Ls, nrows, ro, ko = plan["Ls"], plan["nrows"], plan["ro"], plan["ko"]
    S, SK, NRT = plan["S"], plan["SK"], plan["NRT"]
    nd, na, Lp = plan["nd"], plan["na"], plan["Lp"]
    SP = NRT * 128

    nc = bass.Bass()
    decT = nc.declare_dram_parameter("decT", [HC, 128, S], BF16, isOutput=False)
    senT = nc.declare_dram_parameter("senT", [HC, 128, SK], BF16, isOutput=False)
    Wq = nc.declare_dram_parameter("Wq", [H, H], BF16, isOutput=False)
    Wk = nc.declare_dram_parameter("Wk", [H, H], BF16, isOutput=False)
    bq = nc.declare_dram_parameter("bq", [H], F32, isOutput=False)
    bk = nc.declare_dram_parameter("bk", [H], F32, isOutput=False)
    wt_rep = nc.declare_dram_parameter("wt_rep", [HC, 128, 128], BF16, isOutput=False)
    rowmaskP = nc.declare_dram_parameter("rowmaskP", [SP, N], F32, isOutput=False)
    onehotP = nc.declare_dram_parameter("onehotP", [SP, N], F32, isOutput=False)
    colmaskTP = nc.declare_dram_parameter("colmaskTP", [128, S], F32, isOutput=False)
    out_row = nc.declare_dram_parameter("out_row", [3, 128, NRT], F32, isOutput=True)
    out_col = nc.declare_dram_parameter("out_col", [2, 128, B], F32, isOutput=True)
    dbg = nc.declare_dram_parameter("dbg", [128, SP], F32, isOutput=True)

    from contextlib import ExitStack

    with SafeTileContext(nc) as tc, ExitStack() as ctx:
        consts = ctx.enter_context(tc.tile_pool(name="consts", bufs=1))
        qk_pool = ctx.enter_context(tc.tile_pool(name="qk", bufs=1))
        epool = ctx.enter_context(tc.tile_pool(name="eraw", bufs=ebufs))
        tpool = ctx.enter_context(tc.tile_pool(name="etanh", bufs=ebufs))
        spool = ctx.enter_context(tc.tile_pool(name="scores", bufs=1))
        mpool = ctx.enter_context(tc.tile_pool(name="masks", bufs=2))
        sfpool = ctx.enter_context(tc.tile_pool(name="sflat", bufs=3))
        vpool = ctx.enter_context(tc.tile_pool(name="vecs", bufs=2))
        ps_proj = ctx.enter_context(tc.tile_pool(name="ps_proj", bufs=2, space="PSUM"))
        ps_mv = ctx.enter_context(tc.tile_pool(name="ps_mv", bufs=2, space="PSUM"))
        ps_tr = ctx.enter_context(tc.tile_pool(name="ps_tr", bufs=2, space="PSUM"))

        # ---- load pre-cast bf16 weights and inputs -------------------
        Wq_bf = consts.tile([128, HC, H], BF16, tag="wq")
        Wk_bf = consts.tile([128, HC, H], BF16, tag="wk")
        decT_bf = consts.tile([128, HC, S], BF16, tag="decTb")
        senT_bf = consts.tile([128, HC, SK], BF16, tag="senTb")
        nc.sync.dma_start(Wq_bf[:], Wq.rearrange("(a p) m -> p a m", p=128))
        nc.sync.dma_start(Wk_bf[:], Wk.rearrange("(a p) m -> p a m", p=128))
        for kc in range(HC):
            nc.sync.dma_start(decT_bf[:, kc, :], decT[kc])
            nc.sync.dma_start(senT_bf[:, kc, :], senT[kc])
        bq_sb = consts.tile([128, HC], F32, tag="bq")
        bk_sb = consts.tile([128, HC], F32, tag="bk")
        nc.sync.dma_start(bq_sb[:], bq.rearrange("(a p) -> p a", p=128))
        nc.sync.dma_start(bk_sb[:], bk.rearrange("(a p) -> p a", p=128))
        # wt replicated across 128 stationary columns (host-built): a single
        # LDWEIGHTS serves whole-tile matvec matmuls whose every output
        # partition carries the same score row.
        wtr_bf = consts.tile([128, HC, 128], BF16, tag="wtrb")
        nc.sync.dma_start(wtr_bf[:], wt_rep.rearrange("a p c -> p a c"))

        # ---- projections ---------------------------------------------
        qT = qk_pool.tile([128, HC, S], F32, tag="qT")
        kT = qk_pool.tile([128, HC, SK], BF16, tag="kT")
        for W_bf, xT_bf, b_sb, oT, NC_ in (
            (Wq_bf, decT_bf, bq_sb, qT, S),
            (Wk_bf, senT_bf, bk_sb, kT, SK),
        ):
            for mc in range(HC):
                for n0 in range(0, NC_, 512):
                    nn = min(512, NC_ - n0)
                    pp = ps_proj.tile([128, 512], F32, tag="proj")
                    for kc in range(HC):
                        nc.tensor.matmul(
                            pp[:, :nn],
                            W_bf[:, kc, mc * 128 : (mc + 1) * 128],
                            xT_bf[:, kc, n0 : n0 + nn],
                            start=(kc == 0),
                            stop=(kc == HC - 1),
                        )
                    nc.vector.tensor_scalar(
                        out=oT[:, mc, n0 : n0 + nn], in0=pp[:, :nn],
                        scalar1=b_sb[:, mc : mc + 1], scalar2=None,
                        op0=mybir.AluOpType.add,
                    )

        # ---- big stage ------------------------------------------------
        from concourse.masks import make_identity
        ident = consts.tile([128, 128], F32, tag="ident")
        make_identity(nc, ident)

        # scoresRP[:, rt, :]: packed score rows (row s at partition s%128,
        # tile s//128); filled by per-row DMAs out of the replicated-wt
        # matvec results.
        scoresRP = spool.tile([128, NRT, 128], F32, tag="scoresRP")
        nc.vector.memset(scoresRP[:], 0.0)
        ncopy = 0
        for b in range(B):
            Lpb, nt, ndb = Lp[b], nrows[b], nd[b]
            rob, kob = int(ro[b]), int(ko[b])
            W = nt * Lpb
            etanh = tpool.tile([128, HC, W], BF16, tag="etanh")
            if ndb > 0:
                Wd = ndb * Lpb
                eraw = epool.tile([128, HC, Wd], BF16, tag="eraw")
                for kc in range(HC):
                    for ti in range(ndb):
                        nc.vector.tensor_scalar(
                            out=eraw[:, kc, ti * Lpb : (ti + 1) * Lpb],
                            in0=kT[:, kc, kob : kob + Lpb],
                            scalar1=qT[:, kc, rob + ti : rob + ti + 1],
                            scalar2=None, op0=mybir.AluOpType.add,
                        )
                nc.scalar.activation(
                    etanh[:, :, 0:Wd], eraw[:],
                    mybir.ActivationFunctionType.Tanh,
                )
            for kc in range(HC):
                for ti in range(ndb, nt):
                    nc.scalar.activation(
                        etanh[:, kc, ti * Lpb : (ti + 1) * Lpb],
                        kT[:, kc, kob : kob + Lpb],
                        mybir.ActivationFunctionType.Tanh,
                        bias=qT[:, kc, rob + ti : rob + ti + 1],
                        scale=1.0,
                    )
            g = max(1, 512 // Lpb)
            for t0 in range(0, nt, g):
                gg = min(g, nt - t0)
                wn = gg * Lpb
                pmv = ps_mv.tile([128, 512], F32, tag="mv")
                for kc in range(HC):
                    nc.tensor.matmul(
                        pmv[:, :wn],
                        wtr_bf[:, kc, :],
                        etanh[:, kc, t0 * Lpb : t0 * Lpb + wn],
                        start=(kc == 0),
                        stop=(kc == HC - 1),
                    )
                sflat = sfpool.tile([128, 512], F32, tag="sflat")
                if ncopy % 2 == 0:
                    nc.vector.tensor_copy(sflat[:, :wn], pmv[:, :wn])
                else:
                    nc.scalar.copy(sflat[:, :wn], pmv[:, :wn])
                ncopy += 1
                for r in range(gg):
                    s = rob + t0 + r
                    p, rt = s % 128, s // 128
                    nc.sync.dma_start(
                        scoresRP[p : p + 1, rt, 0:Lpb],
                        sflat[p : p + 1, r * Lpb : r * Lpb + Lpb],
                    )

        # scoresT[j, s] via PE transpose of the packed row tiles
        scoresT = spool.tile([128, SP], F32, tag="scoresT")
        for rt in range(NRT):
            pst = ps_tr.tile([128, 128], F32, tag="tr")
            nc.tensor.transpose(pst[:], scoresRP[:, rt, :], ident[:])
            nc.vector.tensor_copy(scoresT[:, rt * 128 : (rt + 1) * 128], pst[:])

        nc.sync.dma_start(dbg[:], scoresT[:])

        # ---- col softmax partials (per batch, over this core's rows) -
        cmT = mpool.tile([128, S], F32, tag="cmT")
        nc.sync.dma_start(cmT[:], colmaskTP[:])
        cmadd = spool.tile([128, S], F32, tag="cmadd")
        nc.vector.tensor_tensor(out=cmadd[:], in0=scoresT[:, :S], in1=cmT[:],
                                op=mybir.AluOpType.add)
        negm2P = vpool.tile([128, B], F32, tag="negm2P")
        s2P = vpool.tile([128, B], F32, tag="s2P")
        escr = spool.tile([128, 16], BF16, tag="escr")
        for b in range(B):
            nt, rob = nrows[b], int(ro[b])
            nc.vector.tensor_reduce(
                out=negm2P[:, b : b + 1], in_=cmadd[:, rob : rob + nt],
                axis=mybir.AxisListType.X, op=mybir.AluOpType.max, negate=True,
            )
            nc.scalar.activation(
                escr[:, :nt], cmadd[:, rob : rob + nt],
                mybir.ActivationFunctionType.Exp,
                bias=negm2P[:, b : b + 1], scale=1.0,
                accum_out=s2P[:, b : b + 1],
            )
        nc.sync.dma_start(out_col[0], negm2P[:])
        nc.sync.dma_start(out_col[1], s2P[:])

        # ---- row softmax (packed rows, per 128-row tile) -------------
        negm1P = vpool.tile([128, NRT], F32, tag="negm1P")
        s1P = vpool.tile([128, NRT], F32, tag="s1P")
        gscP = vpool.tile([128, NRT], F32, tag="gscP")
        for rt in range(NRT):
            scoresR = scoresRP[:, rt, :]
            rm = mpool.tile([128, N], F32, tag="rm")
            nc.sync.dma_start(rm[:], rowmaskP[rt * 128 : (rt + 1) * 128, :])
            radd = spool.tile([128, N], F32, tag="radd")
            nc.vector.tensor_tensor(out=radd[:], in0=scoresR, in1=rm[:],
                                    op=mybir.AluOpType.add)
            nc.vector.tensor_reduce(
                out=negm1P[:, rt : rt + 1], in_=radd[:],
                axis=mybir.AxisListType.X, op=mybir.AluOpType.max, negate=True,
            )
            escr2 = spool.tile([128, N], BF16, tag="escr2")
            nc.scalar.activation(
                escr2[:], radd[:], mybir.ActivationFunctionType.Exp,
                bias=negm1P[:, rt : rt + 1], scale=1.0,
                accum_out=s1P[:, rt : rt + 1],
            )
            oh = mpool.tile([128, N], F32, tag="oh")
            nc.sync.dma_start(oh[:], onehotP[rt * 128 : (rt + 1) * 128, :])
            gm = spool.tile([128, N], F32, tag="gm")
            nc.vector.tensor_tensor(out=gm[:], in0=scoresR, in1=oh[:],
                                    op=mybir.AluOpType.mult)
            nc.vector.tensor_reduce(
                out=gscP[:, rt : rt + 1], in_=gm[:],
                axis=mybir.AxisListType.X, op=mybir.AluOpType.add,
            )
        nc.sync.dma_start(out_row[0], negm1P[:])
        nc.sync.dma_start(out_row[1], s1P[:])
        nc.sync.dma_start(out_row[2], gscP[:])

    _split_waits(nc, maxw=1)
    return nc


_CACHE2 = {}


def _get_program_v2(plan):
    key = tuple(plan["Ls"])
    if key not in _CACHE2:
        try:
            _CACHE2[key] = _build_program_v2(plan, ebufs=3)
        except Exception:
            # SBUF pressure fallback for large valid regions
            _CACHE2[key] = _build_program_v2(plan, ebufs=2)
    return _CACHE2[key]


def host_prep_v2(dec_outputs, sen_vec, Wq, bq, Wk, bk, wt, bt, target, tgt_len):
    dec_outputs = np.ascontiguousarray(dec_outputs, dtype=np.float32)
    sen_vec = np.ascontiguousarray(sen_vec, dtype=np.float32)
    Wq = np.ascontiguousarray(Wq, dtype=np.float32)
    bq = np.ascontiguousarray(bq, dtype=np.float32)
    Wk = np.ascontiguousarray(Wk, dtype=np.float32)
    bk = np.ascontiguousarray(bk, dtype=np.float32)
    wt = np.ascontiguousarray(wt, dtype=np.float32)
    bt = np.ascontiguousarray(bt, dtype=np.float32)
    target = np.ascontiguousarray(target, dtype=np.int32)
    tgt_len = np.ascontiguousarray(tgt_len, dtype=np.int32)

    plan = _plan(tgt_len)
    Ls, nrows, ro, ko = plan["Ls"], plan["nrows"], plan["ro"], plan["ko"]
    S, SK, NRT, Lp = plan["S"], plan["SK"], plan["NRT"], plan["Lp"]
    SP = NRT * 128

    # masks in global coordinates
    ar = np.arange(N)
    oh_g = (target[..., None] == ar[None, None, :]).astype(np.float32)
    cum = np.cumsum(oh_g, axis=1)
    pointed = np.concatenate([np.zeros_like(cum[:, :1]), cum[:, :-1]], axis=1) > 0
    validj = ar[None, :] < tgt_len[:, None]
    row_m = np.where(pointed | ~validj[:, None, :], NEG, np.float32(0)).astype(np.float32)
    col_m = np.where(~(validj[:, None, :] & validj[:, :, None]), NEG, np.float32(0)).astype(np.float32)

    # per-core packing
    in_maps = []
    rows_of_core = []  # (b, t_global) per packed row s, per core
    for c in range(NCORES):
        tsel = []      # (b, t) for each packed row
        for b in range(B):
            for i in range(nrows[b]):
                tsel.append((b, c + 8 * i))
        rows_of_core.append(tsel)
        bidx = np.array([b for b, t in tsel])
        tidx = np.array([t for b, t in tsel])

        dec_rows = dec_outputs[bidx, tidx, :]               # [S, H]
        decT_p = np.ascontiguousarray(
            dec_rows.T.reshape(HC, 128, S).astype(ml_dtypes.bfloat16)
        )
        ksel_b = np.concatenate([np.full(Lp[b], b) for b in range(B)])
        ksel_j = np.concatenate(
            [np.minimum(np.arange(Lp[b]), N - 1) for b in range(B)]
        )
        sen_rows = sen_vec[ksel_b, ksel_j, :]               # [SK, H]
        senT_p = np.ascontiguousarray(
            sen_rows.T.reshape(HC, 128, SK).astype(ml_dtypes.bfloat16)
        )

        rowmaskP = np.full((SP, N), NEG, np.float32)
        onehotP = np.zeros((SP, N), np.float32)
        rowmaskP[: S] = row_m[bidx, tidx, :]
        onehotP[: S] = oh_g[bidx, tidx, :]
        colmaskTP = np.empty((128, S), np.float32)
        colmaskTP[:] = col_m[bidx, tidx, :].T               # [j, s]

        wt_rep = np.ascontiguousarray(
            np.broadcast_to(
                wt.reshape(HC, 128, 1).astype(ml_dtypes.bfloat16), (HC, 128, 128)
            )
        )
        in_maps.append(
            dict(
                decT=decT_p, senT=senT_p,
                Wq=np.ascontiguousarray(Wq.astype(ml_dtypes.bfloat16)),
                Wk=np.ascontiguousarray(Wk.astype(ml_dtypes.bfloat16)),
                bq=bq, bk=bk, wt_rep=wt_rep,
                rowmaskP=rowmaskP, onehotP=onehotP,
                colmaskTP=np.ascontiguousarray(colmaskTP),
            )
        )
    aux = dict(
        plan=plan, rows_of_core=rows_of_core, row_m=row_m, col_m=col_m,
        validj=validj, target=target, tgt_len=tgt_len, bt=bt,
    )
    return in_maps, aux


def host_combine_v2(results, aux):
    plan = aux["plan"]
    Ls, nrows, ro = plan["Ls"], plan["nrows"], plan["ro"]
    S, NRT = plan["S"], plan["NRT"]
    target = aux["target"]

    lse_row = np.zeros((B, N), np.float32)
    gsc_g = np.zeros((B, N), np.float32)
    m_part = np.empty((NCORES, 128, B), np.float32)   # col max partials
    s_part = np.empty((NCORES, 128, B), np.float32)
    for c in range(NCORES):
        o_row = results[c]["out_row"]                 # [3, 128, NRT]
        o_col = results[c]["out_col"]                 # [2, 128, B]
        tsel = aux["rows_of_core"][c]
        s_idx = np.arange(len(tsel))
        p, rt = s_idx % 128, s_idx // 128
        negm1 = o_row[0, p, rt]
        s1 = o_row[1, p, rt]
        gsc = o_row[2, p, rt]
        with np.errstate(divide="ignore"):
            lse = (-negm1 + np.log(s1)).astype(np.float32)
        bidx = np.array([b for b, t in tsel])
        tidx = np.array([t for b, t in tsel])
        ok = tidx < np.array([Ls[b] for b in bidx])   # ignore padding rows
        lse_row[bidx[ok], tidx[ok]] = lse[ok]
        gsc_g[bidx[ok], tidx[ok]] = gsc[ok]
        m_part[c] = -o_col[0]
        s_part[c] = o_col[1]

    M = m_part.max(axis=0)                            # [128, B]
    with np.errstate(invalid="ignore"):
        sc = (s_part * np.exp(m_part - M[None])).sum(axis=0)
    with np.errstate(divide="ignore"):
        lse_col = (M + np.log(sc)).T.astype(np.float32)  # [B, j]

    bt0 = np.float32(aux["bt"][0])
    lse_row = (lse_row + bt0).astype(np.float32)
    lse_col = (lse_col + bt0).astype(np.float32)

    bi = np.arange(B)[:, None]
    ti = np.arange(N)[None, :]
    g_bt = (gsc_g + bt0).astype(np.float32)
    row_m_at = aux["row_m"][bi, ti, target]
    col_m_at = aux["col_m"][bi, ti, target]
    e_row_at = np.where(row_m_at == 0, g_bt, NEG).astype(np.float32)
    e_col_at = np.where(col_m_at == 0, g_bt, NEG).astype(np.float32)
    lse_col_at = lse_col[bi, target].astype(np.float32)

    validt = aux["validj"]
    nll = np.where(validt, lse_row - e_row_at, np.float32(0)).astype(np.float32)
    nll2 = np.where(validt, lse_col_at - e_col_at, np.float32(0)).astype(np.float32)

    lens = aux["tgt_len"].astype(np.float32)
    d1 = (lens + np.float32(1e-20) - np.float32(1.0)).astype(np.float32)
    row_loss = np.float32(np.mean((nll.sum(axis=1) / d1).astype(np.float32)))
    col_loss = np.float32(np.mean((nll2.sum(axis=1) / (lens * d1)).astype(np.float32)))
    return np.asarray(row_loss + col_loss, dtype=np.float32)


def kernel(dec_outputs, sen_vec, Wq, bq, Wk, bk, wt, bt, target, tgt_len):
    in_maps, aux = host_prep_v2(
        dec_outputs, sen_vec, Wq, bq, Wk, bk, wt, bt, target, tgt_len
    )
    nc = _get_program_v2(aux["plan"])
    res = run_bass_kernel_spmd(nc, in_maps, core_ids=list(range(NCORES)))
    return host_combine_v2(res.results, aux)


# revision 9
# speedup vs baseline: 1.0416x; 1.0416x over previous
"""Trainium2 Bass kernel for the nn_BertForOrdering pointer-network loss.

Row-interleaved valid-region kernel.

Sharding: core c handles rows t ≡ c (mod 8) of EVERY batch element, but
only t < ceil(L_b/8)*8 and columns j < L_b (the valid region — masked
entries of the score matrix never affect the loss beyond their exact -1e9
count, which the host reproduces).  All 8 cores run the same program
(uniform shapes; per-core data differs only in DRAM contents).  Column
softmax is computed as per-core partials (max, sumexp) and combined on
the host; row softmax rows live entirely on one core.
"""

import ml_dtypes
import numpy as np

import bass_rust
import concourse.bass as bass
import concourse.tile as tile
from concourse import mybir
from concourse.bass_utils import run_bass_kernel_spmd
from concourse.vector_clock import ScopedClock

class SafeTileContext(tile.TileContext):
    """Splits the tail-drain's sem waits into 1-wait carrier instructions:
    the walrus build in this container caps sync-wait commands per
    instruction at 1."""

    MAXW = 1

    def _drain_and_barrier(self, tick_clock, wait_clock):
        nc = self.nc
        drain_inst = nc.sync.drain()
        wait_clock.add_sem_waits(
            drain_inst.ins, ScopedClock({None: tick_clock.global_clock})
        )
        si = drain_inst.ins.sync_info
        if si is not None and len(si.on_wait) > self.MAXW:
            waits = list(si.on_wait)
            drain_inst.ins.sync_info = bass_rust.SyncInfo(
                on_wait=waits[: self.MAXW], on_update=list(si.on_update)
            )
            for i in range(self.MAXW, len(waits), self.MAXW):
                extra = nc.sync.drain()
                extra.ins.sync_info = bass_rust.SyncInfo(
                    on_wait=waits[i : i + self.MAXW], on_update=[]
                )
        nc.all_engine_barrier()
        assert self.sems is not None
        popped = nc._tile_sem_poison_stack.pop()
        assert popped is self._sem_poison
        nc.clear_and_free_semaphores(list(self.sems.allocated().values()))
        nc.all_engine_barrier()


def _split_waits(nc, maxw=1):
    """Move excess sync waits onto NOP carriers inserted immediately before
    the instruction in block order (same engine stream -> same semantics)."""

    def carrier(engine):
        bi = nc.engines[engine].nop(nofuse=True)
        ins = bi.ins
        for bb in nc.main_func.blocks:
            lst = bb.instructions
            if lst and lst[-1] is ins:
                lst.pop()
                break
        return ins

    for bb in nc.main_func.blocks:
        lst = bb.instructions
        new = []
        for ins in lst:
            si = ins.sync_info
            if si is not None and len(si.on_wait) > maxw:
                waits = list(si.on_wait)
                keep = waits[-maxw:]
                extra = waits[:-maxw]
                for k in range(0, len(extra), maxw):
                    nop = carrier(ins.engine)
                    nop.sync_info = bass_rust.SyncInfo(
                        on_wait=extra[k : k + maxw], on_update=[]
                    )
                    new.append(nop)
                ins.sync_info = bass_rust.SyncInfo(
                    on_wait=keep, on_update=list(si.on_update)
                )
            new.append(ins)
        lst[:] = new



B, N, H = 16, 128, 768
NCORES = 8
HC = H // 128
NEG = np.float32(-1e9)
F32 = mybir.dt.float32
BF16 = mybir.dt.bfloat16


def _plan(Ls):
    """Static schedule derived from tgt_len values (same on every core)."""
    Ls = [int(x) for x in Ls]
    nrows = [-(-L // 8) for L in Ls]
    Lp = [L + (L & 1) for L in Ls]   # even widths: keeps bf16 DVE in 2x mode
    ro = np.concatenate([[0], np.cumsum(nrows)]).astype(int)  # row offsets
    ko = np.concatenate([[0], np.cumsum(Lp)]).astype(int)     # kT col offsets
    S = int(ro[-1])
    SK = int(ko[-1])
    NRT = -(-S // 128)
    # balance: move trailing rows (t-units) of large-L batches from the
    # DVE-add path to the ACT bias-tanh path until engine times equalize
    dve = 13000.0 + sum(
        6 * (93 + Lp[b] / 2) / 0.96 for b in range(B) for _ in range(nrows[b])
    )
    act = (
        sum(6 * nrows[b] * Lp[b] / 1.2 for b in range(B))
        + 16 * 352 / 1.2
        + 25000.0  # exp + misc + psum copies (ACT trails; keep it lighter)
    )
    na = [0] * B
    units = sorted(
        [(Lp[b], b) for b in range(B) for _ in range(nrows[b])], reverse=True
    )
    for L, b in units:
        save = 6 * (93 + L / 2) / 0.96
        cost = 6 * 352 / 1.2
        if dve > act + save:
            na[b] += 1
            dve -= save
            act += cost
        else:
            break
    nd = [nrows[b] - na[b] for b in range(B)]
    return dict(
        Ls=Ls, Lp=Lp, nrows=nrows, ro=ro, ko=ko, S=S, SK=SK, NRT=NRT, nd=nd, na=na
    )


def _build_program_v2(plan, ebufs=3):
    Ls, nrows, ro, ko = plan["Ls"], plan["nrows"], plan["ro"], plan["ko"]
    S, SK, NRT = plan["S"], plan["SK"], plan["NRT"]
    nd, na, Lp = plan["nd"], plan["na"], plan["Lp"]
    SP = NRT * 128

    nc = bass.Bass()
    decT = nc.declare_dram_parameter("decT", [HC, 128, S], BF16, isOutput=False)
    senT = nc.declare_dram_parameter("senT", [HC, 128, SK], BF16, isOutput=False)
    Wq = nc.declare_dram_parameter("Wq", [H, H], BF16, isOutput=False)
    Wk = nc.declare_dram_parameter("Wk", [H, H], BF16, isOutput=False)
    bq = nc.declare_dram_parameter("bq", [H], F32, isOutput=False)
    bk = nc.declare_dram_parameter("bk", [H], F32, isOutput=False)
    wt_rep = nc.declare_dram_parameter("wt_rep", [HC, 128, 128], BF16, isOutput=False)
    rowmaskP = nc.declare_dram_parameter("rowmaskP", [SP, N], F32, isOutput=False)
    onehotP = nc.declare_dram_parameter("onehotP", [SP, N], F32, isOutput=False)
    colmaskTP = nc.declare_dram_parameter("colmaskTP", [128, S], F32, isOutput=False)
    out_row = nc.declare_dram_parameter("out_row", [3, 128, NRT], F32, isOutput=True)
    out_col = nc.declare_dram_parameter("out_col", [2, 128, B], F32, isOutput=True)

    from contextlib import ExitStack

    with SafeTileContext(nc) as tc, ExitStack() as ctx:
        consts = ctx.enter_context(tc.tile_pool(name="consts", bufs=1))
        qk_pool = ctx.enter_context(tc.tile_pool(name="qk", bufs=1))
        epool = ctx.enter_context(tc.tile_pool(name="eraw", bufs=ebufs))
        tpool = ctx.enter_context(tc.tile_pool(name="etanh", bufs=ebufs))
        spool = ctx.enter_context(tc.tile_pool(name="scores", bufs=1))
        mpool = ctx.enter_context(tc.tile_pool(name="masks", bufs=2))
        sfpool = ctx.enter_context(tc.tile_pool(name="sflat", bufs=3))
        vpool = ctx.enter_context(tc.tile_pool(name="vecs", bufs=2))
        ps_proj = ctx.enter_context(tc.tile_pool(name="ps_proj", bufs=2, space="PSUM"))
        ps_mv = ctx.enter_context(tc.tile_pool(name="ps_mv", bufs=2, space="PSUM"))
        ps_tr = ctx.enter_context(tc.tile_pool(name="ps_tr", bufs=2, space="PSUM"))

        # ---- load pre-cast bf16 weights and inputs -------------------
        Wq_bf = consts.tile([128, HC, H], BF16, tag="wq")
        Wk_bf = consts.tile([128, HC, H], BF16, tag="wk")
        decT_bf = consts.tile([128, HC, S], BF16, tag="decTb")
        senT_bf = consts.tile([128, HC, SK], BF16, tag="senTb")
        nc.sync.dma_start(Wq_bf[:], Wq.rearrange("(a p) m -> p a m", p=128))
        nc.sync.dma_start(Wk_bf[:], Wk.rearrange("(a p) m -> p a m", p=128))
        for kc in range(HC):
            nc.sync.dma_start(decT_bf[:, kc, :], decT[kc])
            nc.sync.dma_start(senT_bf[:, kc, :], senT[kc])
        bq_sb = consts.tile([128, HC], F32, tag="bq")
        bk_sb = consts.tile([128, HC], F32, tag="bk")
        nc.sync.dma_start(bq_sb[:], bq.rearrange("(a p) -> p a", p=128))
        nc.sync.dma_start(bk_sb[:], bk.rearrange("(a p) -> p a", p=128))
        # wt replicated across 128 stationary columns (host-built): a single
        # LDWEIGHTS serves whole-tile matvec matmuls whose every output
        # partition carries the same score row.
        wtr_bf = consts.tile([128, HC, 128], BF16, tag="wtrb")
        nc.sync.dma_start(wtr_bf[:], wt_rep.rearrange("a p c -> p a c"))

        # ---- projections ---------------------------------------------
        qT = qk_pool.tile([128, HC, S], F32, tag="qT")
        kT = qk_pool.tile([128, HC, SK], BF16, tag="kT")
        for W_bf, xT_bf, b_sb, oT, NC_ in (
            (Wq_bf, decT_bf, bq_sb, qT, S),
            (Wk_bf, senT_bf, bk_sb, kT, SK),
        ):
            for mc in range(HC):
                for n0 in range(0, NC_, 512):
                    nn = min(512, NC_ - n0)
                    pp = ps_proj.tile([128, 512], F32, tag="proj")
                    for kc in range(HC):
                        nc.tensor.matmul(
                            pp[:, :nn],
                            W_bf[:, kc, mc * 128 : (mc + 1) * 128],
                            xT_bf[:, kc, n0 : n0 + nn],
                            start=(kc == 0),
                            stop=(kc == HC - 1),
                        )
                    nc.vector.tensor_scalar(
                        out=oT[:, mc, n0 : n0 + nn], in0=pp[:, :nn],
                        scalar1=b_sb[:, mc : mc + 1], scalar2=None,
                        op0=mybir.AluOpType.add,
                    )

        # ---- big stage ------------------------------------------------
        from concourse.masks import make_identity
        ident = consts.tile([128, 128], F32, tag="ident")
        make_identity(nc, ident)

        # scoresRP[:, rt, :]: packed score rows (row s at partition s%128,
        # tile s//128); filled by per-row DMAs out of the replicated-wt
        # matvec results.
        scoresRP = spool.tile([128, NRT, 128], F32, tag="scoresRP")
        nc.vector.memset(scoresRP[:], 0.0)
        ncopy = 0
        border = sorted(range(B), key=lambda b: (-na[b], -nrows[b] * Lp[b]))
        for b in border:
            Lpb, nt, ndb = Lp[b], nrows[b], nd[b]
            rob, kob = int(ro[b]), int(ko[b])
            W = nt * Lpb
            etanh = tpool.tile([128, HC, W], BF16, tag="etanh")
            if ndb > 0:
                Wd = ndb * Lpb
                eraw = epool.tile([128, HC, Wd], BF16, tag="eraw")
                for kc in range(HC):
                    for ti in range(ndb):
                        nc.vector.tensor_scalar(
                            out=eraw[:, kc, ti * Lpb : (ti + 1) * Lpb],
                            in0=kT[:, kc, kob : kob + Lpb],
                            scalar1=qT[:, kc, rob + ti : rob + ti + 1],
                            scalar2=None, op0=mybir.AluOpType.add,
                        )
                nc.scalar.activation(
                    etanh[:, :, 0:Wd], eraw[:],
                    mybir.ActivationFunctionType.Tanh,
                )
            for kc in range(HC):
                for ti in range(ndb, nt):
                    nc.scalar.activation(
                        etanh[:, kc, ti * Lpb : (ti + 1) * Lpb],
                        kT[:, kc, kob : kob + Lpb],
                        mybir.ActivationFunctionType.Tanh,
                        bias=qT[:, kc, rob + ti : rob + ti + 1],
                        scale=1.0,
                    )
            g = max(1, 512 // Lpb)
            for t0 in range(0, nt, g):
                gg = min(g, nt - t0)
                wn = gg * Lpb
                pmv = ps_mv.tile([128, 512], F32, tag="mv")
                for kc in range(HC):
                    nc.tensor.matmul(
                        pmv[:, :wn],
                        wtr_bf[:, kc, :],
                        etanh[:, kc, t0 * Lpb : t0 * Lpb + wn],
                        start=(kc == 0),
                        stop=(kc == HC - 1),
                    )
                sflat = sfpool.tile([128, 512], F32, tag="sflat")
                if ncopy % 3 != 2:
                    nc.vector.tensor_copy(sflat[:, :wn], pmv[:, :wn])
                else:
                    nc.scalar.copy(sflat[:, :wn], pmv[:, :wn])
                ncopy += 1
                for r in range(gg):
                    s = rob + t0 + r
                    p, rt = s % 128, s // 128
                    nc.sync.dma_start(
                        scoresRP[p : p + 1, rt, 0:Lpb],
                        sflat[p : p + 1, r * Lpb : r * Lpb + Lpb],
                    )

        # scoresT[j, s] via PE transpose of the packed row tiles
        scoresT = spool.tile([128, SP], F32, tag="scoresT")
        for rt in range(NRT):
            pst = ps_tr.tile([128, 128], F32, tag="tr")
            nc.tensor.transpose(pst[:], scoresRP[:, rt, :], ident[:])
            nc.vector.tensor_copy(scoresT[:, rt * 128 : (rt + 1) * 128], pst[:])

        # ---- col softmax partials (per batch, over this core's rows) -
        cmT = mpool.tile([128, S], F32, tag="cmT")
        nc.sync.dma_start(cmT[:], colmaskTP[:])
        cmadd = spool.tile([128, S], F32, tag="cmadd")
        nc.vector.tensor_tensor(out=cmadd[:], in0=scoresT[:, :S], in1=cmT[:],
                                op=mybir.AluOpType.add)
        negm2P = vpool.tile([128, B], F32, tag="negm2P")
        s2P = vpool.tile([128, B], F32, tag="s2P")
        escr = spool.tile([128, 16], BF16, tag="escr")
        for b in range(B):
            nt, rob = nrows[b], int(ro[b])
            nc.vector.tensor_reduce(
                out=negm2P[:, b : b + 1], in_=cmadd[:, rob : rob + nt],
                axis=mybir.AxisListType.X, op=mybir.AluOpType.max, negate=True,
            )
            nc.scalar.activation(
                escr[:, :nt], cmadd[:, rob : rob + nt],
                mybir.ActivationFunctionType.Exp,
                bias=negm2P[:, b : b + 1], scale=1.0,
                accum_out=s2P[:, b : b + 1],
            )
        nc.sync.dma_start(out_col[0], negm2P[:])
        nc.sync.dma_start(out_col[1], s2P[:])

        # ---- row softmax (packed rows, per 128-row tile) -------------
        negm1P = vpool.tile([128, NRT], F32, tag="negm1P")
        s1P = vpool.tile([128, NRT], F32, tag="s1P")
        gscP = vpool.tile([128, NRT], F32, tag="gscP")
        for rt in range(NRT):
            scoresR = scoresRP[:, rt, :]
            rm = mpool.tile([128, N], F32, tag="rm")
            nc.sync.dma_start(rm[:], rowmaskP[rt * 128 : (rt + 1) * 128, :])
            radd = spool.tile([128, N], F32, tag="radd")
            nc.vector.tensor_tensor(out=radd[:], in0=scoresR, in1=rm[:],
                                    op=mybir.AluOpType.add)
            nc.vector.tensor_reduce(
                out=negm1P[:, rt : rt + 1], in_=radd[:],
                axis=mybir.AxisListType.X, op=mybir.AluOpType.max, negate=True,
            )
            escr2 = spool.tile([128, N], BF16, tag="escr2")
            nc.scalar.activation(
                escr2[:], radd[:], mybir.ActivationFunctionType.Exp,
                bias=negm1P[:, rt : rt + 1], scale=1.0,
                accum_out=s1P[:, rt : rt + 1],
            )
            oh = mpool.tile([128, N], F32, tag="oh")
            nc.sync.dma_start(oh[:], onehotP[rt * 128 : (rt + 1) * 128, :])
            gm = spool.tile([128, N], F32, tag="gm")
            nc.vector.tensor_tensor(out=gm[:], in0=scoresR, in1=oh[:],
                                    op=mybir.AluOpType.mult)
            nc.vector.tensor_reduce(
                out=gscP[:, rt : rt + 1], in_=gm[:],
                axis=mybir.AxisListType.X, op=mybir.AluOpType.add,
            )
        nc.sync.dma_start(out_row[0], negm1P[:])
        nc.sync.dma_start(out_row[1], s1P[:])
        nc.sync.dma_start(out_row[2], gscP[:])

    _split_waits(nc, maxw=1)
    return nc


_CACHE2 = {}


def _get_program_v2(plan):
    key = tuple(plan["Ls"])
    if key not in _CACHE2:
        try:
            _CACHE2[key] = _build_program_v2(plan, ebufs=3)
        except Exception:
            # SBUF pressure fallback for large valid regions
            _CACHE2[key] = _build_program_v2(plan, ebufs=2)
    return _CACHE2[key]


def host_prep_v2(dec_outputs, sen_vec, Wq, bq, Wk, bk, wt, bt, target, tgt_len):
    dec_outputs = np.ascontiguousarray(dec_outputs, dtype=np.float32)
    sen_vec = np.ascontiguousarray(sen_vec, dtype=np.float32)
    Wq = np.ascontiguousarray(Wq, dtype=np.float32)
    bq = np.ascontiguousarray(bq, dtype=np.float32)
    Wk = np.ascontiguousarray(Wk, dtype=np.float32)
    bk = np.ascontiguousarray(bk, dtype=np.float32)
    wt = np.ascontiguousarray(wt, dtype=np.float32)
    bt = np.ascontiguousarray(bt, dtype=np.float32)
    target = np.ascontiguousarray(target, dtype=np.int32)
    tgt_len = np.ascontiguousarray(tgt_len, dtype=np.int32)

    plan = _plan(tgt_len)
    Ls, nrows, ro, ko = plan["Ls"], plan["nrows"], plan["ro"], plan["ko"]
    S, SK, NRT, Lp = plan["S"], plan["SK"], plan["NRT"], plan["Lp"]
    SP = NRT * 128

    # masks in global coordinates
    ar = np.arange(N)
    oh_g = (target[..., None] == ar[None, None, :]).astype(np.float32)
    cum = np.cumsum(oh_g, axis=1)
    pointed = np.concatenate([np.zeros_like(cum[:, :1]), cum[:, :-1]], axis=1) > 0
    validj = ar[None, :] < tgt_len[:, None]
    row_m = np.where(pointed | ~validj[:, None, :], NEG, np.float32(0)).astype(np.float32)
    col_m = np.where(~(validj[:, None, :] & validj[:, :, None]), NEG, np.float32(0)).astype(np.float32)

    # per-core packing
    in_maps = []
    rows_of_core = []  # (b, t_global) per packed row s, per core
    for c in range(NCORES):
        tsel = []      # (b, t) for each packed row
        for b in range(B):
            for i in range(nrows[b]):
                tsel.append((b, c + 8 * i))
        rows_of_core.append(tsel)
        bidx = np.array([b for b, t in tsel])
        tidx = np.array([t for b, t in tsel])

        dec_rows = dec_outputs[bidx, tidx, :]               # [S, H]
        decT_p = np.ascontiguousarray(
            dec_rows.T.reshape(HC, 128, S).astype(ml_dtypes.bfloat16)
        )
        ksel_b = np.concatenate([np.full(Lp[b], b) for b in range(B)])
        ksel_j = np.concatenate(
            [np.minimum(np.arange(Lp[b]), N - 1) for b in range(B)]
        )
        sen_rows = sen_vec[ksel_b, ksel_j, :]               # [SK, H]
        senT_p = np.ascontiguousarray(
            sen_rows.T.reshape(HC, 128, SK).astype(ml_dtypes.bfloat16)
        )

        rowmaskP = np.full((SP, N), NEG, np.float32)
        onehotP = np.zeros((SP, N), np.float32)
        rowmaskP[: S] = row_m[bidx, tidx, :]
        onehotP[: S] = oh_g[bidx, tidx, :]
        colmaskTP = np.empty((128, S), np.float32)
        colmaskTP[:] = col_m[bidx, tidx, :].T               # [j, s]

        wt_rep = np.ascontiguousarray(
            np.broadcast_to(
                wt.reshape(HC, 128, 1).astype(ml_dtypes.bfloat16), (HC, 128, 128)
            )
        )
        in_maps.append(
            dict(
                decT=decT_p, senT=senT_p,
                Wq=np.ascontiguousarray(Wq.astype(ml_dtypes.bfloat16)),
                Wk=np.ascontiguousarray(Wk.astype(ml_dtypes.bfloat16)),
                bq=bq, bk=bk, wt_rep=wt_rep,
                rowmaskP=rowmaskP, onehotP=onehotP,
                colmaskTP=np.ascontiguousarray(colmaskTP),
            )
        )
    aux = dict(
        plan=plan, rows_of_core=rows_of_core, row_m=row_m, col_m=col_m,
        validj=validj, target=target, tgt_len=tgt_len, bt=bt,
    )
    return in_maps, aux


def host_combine_v2(results, aux):
    plan = aux["plan"]
    Ls, nrows, ro = plan["Ls"], plan["nrows"], plan["ro"]
    S, NRT = plan["S"], plan["NRT"]
    target = aux["target"]

    lse_row = np.zeros((B, N), np.float32)
    gsc_g = np.zeros((B, N), np.float32)
    m_part = np.empty((NCORES, 128, B), np.float32)   # col max partials
    s_part = np.empty((NCORES, 128, B), np.float32)
    for c in range(NCORES):
        o_row = results[c]["out_row"]                 # [3, 128, NRT]
        o_col = results[c]["out_col"]                 # [2, 128, B]
        tsel = aux["rows_of_core"][c]
        s_idx = np.arange(len(tsel))
        p, rt = s_idx % 128, s_idx // 128
        negm1 = o_row[0, p, rt]
        s1 = o_row[1, p, rt]
        gsc = o_row[2, p, rt]
        with np.errstate(divide="ignore"):
            lse = (-negm1 + np.log(s1)).astype(np.float32)
        bidx = np.array([b for b, t in tsel])
        tidx = np.array([t for b, t in tsel])
        ok = tidx < np.array([Ls[b] for b in bidx])   # ignore padding rows
        lse_row[bidx[ok], tidx[ok]] = lse[ok]
        gsc_g[bidx[ok], tidx[ok]] = gsc[ok]
        m_part[c] = -o_col[0]
        s_part[c] = o_col[1]

    M = m_part.max(axis=0)                            # [128, B]
    with np.errstate(invalid="ignore"):
        sc = (s_part * np.exp(m_part - M[None])).sum(axis=0)
    with np.errstate(divide="ignore"):
        lse_col = (M + np.log(sc)).T.astype(np.float32)  # [B, j]

    bt0 = np.float32(aux["bt"][0])
    lse_row = (lse_row + bt0).astype(np.float32)
    lse_col = (lse_col + bt0).astype(np.float32)

    bi = np.arange(B)[:, None]
    ti = np.arange(N)[None, :]
    g_bt = (gsc_g + bt0).astype(np.float32)
    row_m_at = aux["row_m"][bi, ti, target]
    col_m_at = aux["col_m"][bi, ti, target]
    e_row_at = np.where(row_m_at == 0, g_bt, NEG).astype(np.float32)
    e_col_at = np.where(col_m_at == 0, g_bt, NEG).astype(np.float32)
    lse_col_at = lse_col[bi, target].astype(np.float32)

    validt = aux["validj"]
    nll = np.where(validt, lse_row - e_row_at, np.float32(0)).astype(np.float32)
    nll2 = np.where(validt, lse_col_at - e_col_at, np.float32(0)).astype(np.float32)

    lens = aux["tgt_len"].astype(np.float32)
    d1 = (lens + np.float32(1e-20) - np.float32(1.0)).astype(np.float32)
    row_loss = np.float32(np.mean((nll.sum(axis=1) / d1).astype(np.float32)))
    col_loss = np.float32(np.mean((nll2.sum(axis=1) / (lens * d1)).astype(np.float32)))
    return np.asarray(row_loss + col_loss, dtype=np.float32)


def kernel(dec_outputs, sen_vec, Wq, bq, Wk, bk, wt, bt, target, tgt_len):
    in_maps, aux = host_prep_v2(
        dec_outputs, sen_vec, Wq, bq, Wk, bk, wt, bt, target, tgt_len
    )
    nc = _get_program_v2(aux["plan"])
    res = run_bass_kernel_spmd(nc, in_maps, core_ids=list(range(NCORES)))
    return host_combine_v2(res.results, aux)


# revision 10
# speedup vs baseline: 1.0500x; 1.0081x over previous
"""Trainium2 Bass kernel for the nn_BertForOrdering pointer-network loss.

Row-interleaved valid-region kernel.

Sharding: core c handles rows t ≡ c (mod 8) of EVERY batch element, but
only t < ceil(L_b/8)*8 and columns j < L_b (the valid region — masked
entries of the score matrix never affect the loss beyond their exact -1e9
count, which the host reproduces).  All 8 cores run the same program
(uniform shapes; per-core data differs only in DRAM contents).  Column
softmax is computed as per-core partials (max, sumexp) and combined on
the host; row softmax rows live entirely on one core.
"""

import ml_dtypes
import numpy as np

import bass_rust
import concourse.bass as bass
import concourse.tile as tile
from concourse import mybir
from concourse.bass_utils import run_bass_kernel_spmd
from concourse.vector_clock import ScopedClock

class SafeTileContext(tile.TileContext):
    """Splits the tail-drain's sem waits into 1-wait carrier instructions:
    the walrus build in this container caps sync-wait commands per
    instruction at 1."""

    MAXW = 1

    def _drain_and_barrier(self, tick_clock, wait_clock):
        nc = self.nc
        drain_inst = nc.sync.drain()
        wait_clock.add_sem_waits(
            drain_inst.ins, ScopedClock({None: tick_clock.global_clock})
        )
        si = drain_inst.ins.sync_info
        if si is not None and len(si.on_wait) > self.MAXW:
            waits = list(si.on_wait)
            drain_inst.ins.sync_info = bass_rust.SyncInfo(
                on_wait=waits[: self.MAXW], on_update=list(si.on_update)
            )
            for i in range(self.MAXW, len(waits), self.MAXW):
                extra = nc.sync.drain()
                extra.ins.sync_info = bass_rust.SyncInfo(
                    on_wait=waits[i : i + self.MAXW], on_update=[]
                )
        nc.all_engine_barrier()
        assert self.sems is not None
        popped = nc._tile_sem_poison_stack.pop()
        assert popped is self._sem_poison
        nc.clear_and_free_semaphores(list(self.sems.allocated().values()))
        nc.all_engine_barrier()


def _split_waits(nc, maxw=1):
    """Move excess sync waits onto NOP carriers inserted immediately before
    the instruction in block order (same engine stream -> same semantics)."""

    def carrier(engine):
        bi = nc.engines[engine].nop(nofuse=True)
        ins = bi.ins
        for bb in nc.main_func.blocks:
            lst = bb.instructions
            if lst and lst[-1] is ins:
                lst.pop()
                break
        return ins

    for bb in nc.main_func.blocks:
        lst = bb.instructions
        new = []
        for ins in lst:
            si = ins.sync_info
            if si is not None and len(si.on_wait) > maxw:
                waits = list(si.on_wait)
                keep = waits[-maxw:]
                extra = waits[:-maxw]
                for k in range(0, len(extra), maxw):
                    nop = carrier(ins.engine)
                    nop.sync_info = bass_rust.SyncInfo(
                        on_wait=extra[k : k + maxw], on_update=[]
                    )
                    new.append(nop)
                ins.sync_info = bass_rust.SyncInfo(
                    on_wait=keep, on_update=list(si.on_update)
                )
            new.append(ins)
        lst[:] = new



B, N, H = 16, 128, 768
NCORES = 8
HC = H // 128
NEG = np.float32(-1e9)
F32 = mybir.dt.float32
BF16 = mybir.dt.bfloat16


def _plan(Ls):
    """Static schedule derived from tgt_len values (same on every core)."""
    Ls = [int(x) for x in Ls]
    nrows = [-(-L // 8) for L in Ls]
    Lp = [L + (L & 1) for L in Ls]   # even widths: keeps bf16 DVE in 2x mode
    ro = np.concatenate([[0], np.cumsum(nrows)]).astype(int)  # row offsets
    ko = np.concatenate([[0], np.cumsum(Lp)]).astype(int)     # kT col offsets
    S = int(ro[-1])
    SK = int(ko[-1])
    NRT = -(-S // 128)
    # balance: move trailing rows (t-units) of large-L batches from the
    # DVE-add path to the ACT bias-tanh path until engine times equalize
    dve = 13000.0 + sum(
        6 * (93 + Lp[b] / 2) / 0.96 for b in range(B) for _ in range(nrows[b])
    )
    act = (
        sum(6 * nrows[b] * Lp[b] / 1.2 for b in range(B))
        + 16 * 352 / 1.2
        + 25000.0  # exp + misc + psum copies (ACT trails; keep it lighter)
    )
    na = [0] * B
    units = sorted(
        [(Lp[b], b) for b in range(B) for _ in range(nrows[b])], reverse=True
    )
    for L, b in units:
        save = 6 * (93 + L / 2) / 0.96
        cost = 6 * 352 / 1.2
        if dve > act + save:
            na[b] += 1
            dve -= save
            act += cost
        else:
            break
    nd = [nrows[b] - na[b] for b in range(B)]
    return dict(
        Ls=Ls, Lp=Lp, nrows=nrows, ro=ro, ko=ko, S=S, SK=SK, NRT=NRT, nd=nd, na=na
    )


def _build_program_v2(plan, ebufs=3):
    Ls, nrows, ro, ko = plan["Ls"], plan["nrows"], plan["ro"], plan["ko"]
    S, SK, NRT = plan["S"], plan["SK"], plan["NRT"]
    nd, na, Lp = plan["nd"], plan["na"], plan["Lp"]
    SP = NRT * 128

    nc = bass.Bass()
    decT = nc.declare_dram_parameter("decT", [HC, 128, S], BF16, isOutput=False)
    senT = nc.declare_dram_parameter("senT", [HC, 128, SK], BF16, isOutput=False)
    Wq = nc.declare_dram_parameter("Wq", [H, H], BF16, isOutput=False)
    Wk = nc.declare_dram_parameter("Wk", [H, H], BF16, isOutput=False)
    bq = nc.declare_dram_parameter("bq", [H], F32, isOutput=False)
    bk = nc.declare_dram_parameter("bk", [H], F32, isOutput=False)
    wt_rep = nc.declare_dram_parameter("wt_rep", [HC, 128, 128], BF16, isOutput=False)
    rowmaskP = nc.declare_dram_parameter("rowmaskP", [SP, N], F32, isOutput=False)
    onehotP = nc.declare_dram_parameter("onehotP", [SP, N], F32, isOutput=False)
    colmaskTP = nc.declare_dram_parameter("colmaskTP", [128, S], F32, isOutput=False)
    out_row = nc.declare_dram_parameter("out_row", [3, 128, NRT], F32, isOutput=True)
    out_col = nc.declare_dram_parameter("out_col", [2, 128, B], F32, isOutput=True)

    from contextlib import ExitStack

    with SafeTileContext(nc) as tc, ExitStack() as ctx:
        consts = ctx.enter_context(tc.tile_pool(name="consts", bufs=1))
        qk_pool = ctx.enter_context(tc.tile_pool(name="qk", bufs=1))
        epool = ctx.enter_context(tc.tile_pool(name="eraw", bufs=ebufs))
        tpool = ctx.enter_context(tc.tile_pool(name="etanh", bufs=ebufs))
        spool = ctx.enter_context(tc.tile_pool(name="scores", bufs=1))
        mpool = ctx.enter_context(tc.tile_pool(name="masks", bufs=2))
        sfpool = ctx.enter_context(tc.tile_pool(name="sflat", bufs=3))
        vpool = ctx.enter_context(tc.tile_pool(name="vecs", bufs=2))
        ps_proj = ctx.enter_context(tc.tile_pool(name="ps_proj", bufs=2, space="PSUM"))
        ps_mv = ctx.enter_context(tc.tile_pool(name="ps_mv", bufs=3, space="PSUM"))
        ps_tr = ctx.enter_context(tc.tile_pool(name="ps_tr", bufs=2, space="PSUM"))

        # ---- load pre-cast bf16 weights and inputs -------------------
        Wq_bf = consts.tile([128, HC, H], BF16, tag="wq")
        Wk_bf = consts.tile([128, HC, H], BF16, tag="wk")
        decT_bf = consts.tile([128, HC, S], BF16, tag="decTb")
        senT_bf = consts.tile([128, HC, SK], BF16, tag="senTb")
        nc.sync.dma_start(Wq_bf[:], Wq.rearrange("(a p) m -> p a m", p=128))
        nc.sync.dma_start(Wk_bf[:], Wk.rearrange("(a p) m -> p a m", p=128))
        for kc in range(HC):
            nc.sync.dma_start(decT_bf[:, kc, :], decT[kc])
            nc.sync.dma_start(senT_bf[:, kc, :], senT[kc])
        bq_sb = consts.tile([128, HC], F32, tag="bq")
        bk_sb = consts.tile([128, HC], F32, tag="bk")
        nc.sync.dma_start(bq_sb[:], bq.rearrange("(a p) -> p a", p=128))
        nc.sync.dma_start(bk_sb[:], bk.rearrange("(a p) -> p a", p=128))
        # wt replicated across 128 stationary columns (host-built): a single
        # LDWEIGHTS serves whole-tile matvec matmuls whose every output
        # partition carries the same score row.
        wtr_bf = consts.tile([128, HC, 128], BF16, tag="wtrb")
        nc.sync.dma_start(wtr_bf[:], wt_rep.rearrange("a p c -> p a c"))

        # ---- projections ---------------------------------------------
        qT = qk_pool.tile([128, HC, S], F32, tag="qT")
        kT = qk_pool.tile([128, HC, SK], BF16, tag="kT")
        for W_bf, xT_bf, b_sb, oT, NC_ in (
            (Wq_bf, decT_bf, bq_sb, qT, S),
            (Wk_bf, senT_bf, bk_sb, kT, SK),
        ):
            for mc in range(HC):
                for n0 in range(0, NC_, 512):
                    nn = min(512, NC_ - n0)
                    pp = ps_proj.tile([128, 512], F32, tag="proj")
                    for kc in range(HC):
                        nc.tensor.matmul(
                            pp[:, :nn],
                            W_bf[:, kc, mc * 128 : (mc + 1) * 128],
                            xT_bf[:, kc, n0 : n0 + nn],
                            start=(kc == 0),
                            stop=(kc == HC - 1),
                        )
                    nc.vector.tensor_scalar(
                        out=oT[:, mc, n0 : n0 + nn], in0=pp[:, :nn],
                        scalar1=b_sb[:, mc : mc + 1], scalar2=None,
                        op0=mybir.AluOpType.add,
                    )

        # ---- big stage ------------------------------------------------
        from concourse.masks import make_identity
        ident = consts.tile([128, 128], F32, tag="ident")
        make_identity(nc, ident)

        # scoresRP[:, rt, :]: packed score rows (row s at partition s%128,
        # tile s//128); filled by per-row DMAs out of the replicated-wt
        # matvec results.
        scoresRP = spool.tile([128, NRT, 128], F32, tag="scoresRP")
        nc.vector.memset(scoresRP[:], 0.0)
        # prefetch softmax-stage masks so the stats tail never waits on DMA
        cmT = mpool.tile([128, S], F32, tag="cmT")
        nc.sync.dma_start(cmT[:], colmaskTP[:])
        rm_t = []
        oh_t = []
        for rt in range(NRT):
            rm = mpool.tile([128, N], F32, tag=f"rm{rt}")
            nc.sync.dma_start(rm[:], rowmaskP[rt * 128 : (rt + 1) * 128, :])
            rm_t.append(rm)
            oh = mpool.tile([128, N], F32, tag=f"oh{rt}")
            nc.sync.dma_start(oh[:], onehotP[rt * 128 : (rt + 1) * 128, :])
            oh_t.append(oh)
        ncopy = 0
        border = sorted(range(B), key=lambda b: (-na[b], -nrows[b] * Lp[b]))
        for b in border:
            Lpb, nt, ndb = Lp[b], nrows[b], nd[b]
            rob, kob = int(ro[b]), int(ko[b])
            W = nt * Lpb
            etanh = tpool.tile([128, HC, W], BF16, tag="etanh")
            if ndb > 0:
                Wd = ndb * Lpb
                eraw = epool.tile([128, HC, Wd], BF16, tag="eraw")
                for kc in range(HC):
                    for ti in range(ndb):
                        nc.vector.tensor_scalar(
                            out=eraw[:, kc, ti * Lpb : (ti + 1) * Lpb],
                            in0=kT[:, kc, kob : kob + Lpb],
                            scalar1=qT[:, kc, rob + ti : rob + ti + 1],
                            scalar2=None, op0=mybir.AluOpType.add,
                        )
                nc.scalar.activation(
                    etanh[:, :, 0:Wd], eraw[:],
                    mybir.ActivationFunctionType.Tanh,
                )
            for kc in range(HC):
                for ti in range(ndb, nt):
                    nc.scalar.activation(
                        etanh[:, kc, ti * Lpb : (ti + 1) * Lpb],
                        kT[:, kc, kob : kob + Lpb],
                        mybir.ActivationFunctionType.Tanh,
                        bias=qT[:, kc, rob + ti : rob + ti + 1],
                        scale=1.0,
                    )
            g = max(1, 512 // Lpb)
            for t0 in range(0, nt, g):
                gg = min(g, nt - t0)
                wn = gg * Lpb
                pmv = ps_mv.tile([128, 512], F32, tag="mv")
                for kc in range(HC):
                    nc.tensor.matmul(
                        pmv[:, :wn],
                        wtr_bf[:, kc, :],
                        etanh[:, kc, t0 * Lpb : t0 * Lpb + wn],
                        start=(kc == 0),
                        stop=(kc == HC - 1),
                    )
                sflat = sfpool.tile([128, 512], F32, tag="sflat")
                if ncopy % 3 != 2:
                    nc.vector.tensor_copy(sflat[:, :wn], pmv[:, :wn])
                else:
                    nc.scalar.copy(sflat[:, :wn], pmv[:, :wn])
                ncopy += 1
                for r in range(gg):
                    s = rob + t0 + r
                    p, rt = s % 128, s // 128
                    nc.sync.dma_start(
                        scoresRP[p : p + 1, rt, 0:Lpb],
                        sflat[p : p + 1, r * Lpb : r * Lpb + Lpb],
                    )

        # scoresT[j, s] via PE transpose of the packed row tiles
        scoresT = spool.tile([128, SP], F32, tag="scoresT")
        for rt in range(NRT):
            pst = ps_tr.tile([128, 128], F32, tag="tr")
            nc.tensor.transpose(pst[:], scoresRP[:, rt, :], ident[:])
            nc.vector.tensor_copy(scoresT[:, rt * 128 : (rt + 1) * 128], pst[:])

        # ---- col softmax partials (per batch, over this core's rows) -
        cmadd = spool.tile([128, S], F32, tag="cmadd")
        nc.vector.tensor_tensor(out=cmadd[:], in0=scoresT[:, :S], in1=cmT[:],
                                op=mybir.AluOpType.add)
        negm2P = vpool.tile([128, B], F32, tag="negm2P")
        s2P = vpool.tile([128, B], F32, tag="s2P")
        escr = spool.tile([128, 16], BF16, tag="escr")
        for b in range(B):
            nt, rob = nrows[b], int(ro[b])
            nc.vector.tensor_reduce(
                out=negm2P[:, b : b + 1], in_=cmadd[:, rob : rob + nt],
                axis=mybir.AxisListType.X, op=mybir.AluOpType.max, negate=True,
            )
            nc.scalar.activation(
                escr[:, :nt], cmadd[:, rob : rob + nt],
                mybir.ActivationFunctionType.Exp,
                bias=negm2P[:, b : b + 1], scale=1.0,
                accum_out=s2P[:, b : b + 1],
            )
        nc.sync.dma_start(out_col[0], negm2P[:])
        nc.sync.dma_start(out_col[1], s2P[:])

        # ---- row softmax (packed rows, per 128-row tile) -------------
        negm1P = vpool.tile([128, NRT], F32, tag="negm1P")
        s1P = vpool.tile([128, NRT], F32, tag="s1P")
        gscP = vpool.tile([128, NRT], F32, tag="gscP")
        for rt in range(NRT):
            scoresR = scoresRP[:, rt, :]
            rm = rm_t[rt]
            radd = spool.tile([128, N], F32, tag="radd")
            nc.vector.tensor_tensor(out=radd[:], in0=scoresR, in1=rm[:],
                                    op=mybir.AluOpType.add)
            nc.vector.tensor_reduce(
                out=negm1P[:, rt : rt + 1], in_=radd[:],
                axis=mybir.AxisListType.X, op=mybir.AluOpType.max, negate=True,
            )
            escr2 = spool.tile([128, N], BF16, tag="escr2")
            nc.scalar.activation(
                escr2[:], radd[:], mybir.ActivationFunctionType.Exp,
                bias=negm1P[:, rt : rt + 1], scale=1.0,
                accum_out=s1P[:, rt : rt + 1],
            )
            oh = oh_t[rt]
            gm = spool.tile([128, N], F32, tag="gm")
            nc.vector.tensor_tensor(out=gm[:], in0=scoresR, in1=oh[:],
                                    op=mybir.AluOpType.mult)
            nc.vector.tensor_reduce(
                out=gscP[:, rt : rt + 1], in_=gm[:],
                axis=mybir.AxisListType.X, op=mybir.AluOpType.add,
            )
        nc.sync.dma_start(out_row[0], negm1P[:])
        nc.sync.dma_start(out_row[1], s1P[:])
        nc.sync.dma_start(out_row[2], gscP[:])

    _split_waits(nc, maxw=1)
    return nc


_CACHE2 = {}


def _get_program_v2(plan):
    key = tuple(plan["Ls"])
    if key not in _CACHE2:
        try:
            _CACHE2[key] = _build_program_v2(plan, ebufs=3)
        except Exception:
            # SBUF pressure fallback for large valid regions
            _CACHE2[key] = _build_program_v2(plan, ebufs=2)
    return _CACHE2[key]


def host_prep_v2(dec_outputs, sen_vec, Wq, bq, Wk, bk, wt, bt, target, tgt_len):
    dec_outputs = np.ascontiguousarray(dec_outputs, dtype=np.float32)
    sen_vec = np.ascontiguousarray(sen_vec, dtype=np.float32)
    Wq = np.ascontiguousarray(Wq, dtype=np.float32)
    bq = np.ascontiguousarray(bq, dtype=np.float32)
    Wk = np.ascontiguousarray(Wk, dtype=np.float32)
    bk = np.ascontiguousarray(bk, dtype=np.float32)
    wt = np.ascontiguousarray(wt, dtype=np.float32)
    bt = np.ascontiguousarray(bt, dtype=np.float32)
    target = np.ascontiguousarray(target, dtype=np.int32)
    tgt_len = np.ascontiguousarray(tgt_len, dtype=np.int32)

    plan = _plan(tgt_len)
    Ls, nrows, ro, ko = plan["Ls"], plan["nrows"], plan["ro"], plan["ko"]
    S, SK, NRT, Lp = plan["S"], plan["SK"], plan["NRT"], plan["Lp"]
    SP = NRT * 128

    # masks in global coordinates
    ar = np.arange(N)
    oh_g = (target[..., None] == ar[None, None, :]).astype(np.float32)
    cum = np.cumsum(oh_g, axis=1)
    pointed = np.concatenate([np.zeros_like(cum[:, :1]), cum[:, :-1]], axis=1) > 0
    validj = ar[None, :] < tgt_len[:, None]
    row_m = np.where(pointed | ~validj[:, None, :], NEG, np.float32(0)).astype(np.float32)
    col_m = np.where(~(validj[:, None, :] & validj[:, :, None]), NEG, np.float32(0)).astype(np.float32)

    # per-core packing
    in_maps = []
    rows_of_core = []  # (b, t_global) per packed row s, per core
    for c in range(NCORES):
        tsel = []      # (b, t) for each packed row
        for b in range(B):
            for i in range(nrows[b]):
                tsel.append((b, c + 8 * i))
        rows_of_core.append(tsel)
        bidx = np.array([b for b, t in tsel])
        tidx = np.array([t for b, t in tsel])

        dec_rows = dec_outputs[bidx, tidx, :]               # [S, H]
        decT_p = np.ascontiguousarray(
            dec_rows.T.reshape(HC, 128, S).astype(ml_dtypes.bfloat16)
        )
        ksel_b = np.concatenate([np.full(Lp[b], b) for b in range(B)])
        ksel_j = np.concatenate(
            [np.minimum(np.arange(Lp[b]), N - 1) for b in range(B)]
        )
        sen_rows = sen_vec[ksel_b, ksel_j, :]               # [SK, H]
        senT_p = np.ascontiguousarray(
            sen_rows.T.reshape(HC, 128, SK).astype(ml_dtypes.bfloat16)
        )

        rowmaskP = np.full((SP, N), NEG, np.float32)
        onehotP = np.zeros((SP, N), np.float32)
        rowmaskP[: S] = row_m[bidx, tidx, :]
        onehotP[: S] = oh_g[bidx, tidx, :]
        colmaskTP = np.empty((128, S), np.float32)
        colmaskTP[:] = col_m[bidx, tidx, :].T               # [j, s]

        wt_rep = np.ascontiguousarray(
            np.broadcast_to(
                wt.reshape(HC, 128, 1).astype(ml_dtypes.bfloat16), (HC, 128, 128)
            )
        )
        in_maps.append(
            dict(
                decT=decT_p, senT=senT_p,
                Wq=np.ascontiguousarray(Wq.astype(ml_dtypes.bfloat16)),
                Wk=np.ascontiguousarray(Wk.astype(ml_dtypes.bfloat16)),
                bq=bq, bk=bk, wt_rep=wt_rep,
                rowmaskP=rowmaskP, onehotP=onehotP,
                colmaskTP=np.ascontiguousarray(colmaskTP),
            )
        )
    aux = dict(
        plan=plan, rows_of_core=rows_of_core, row_m=row_m, col_m=col_m,
        validj=validj, target=target, tgt_len=tgt_len, bt=bt,
    )
    return in_maps, aux


def host_combine_v2(results, aux):
    plan = aux["plan"]
    Ls, nrows, ro = plan["Ls"], plan["nrows"], plan["ro"]
    S, NRT = plan["S"], plan["NRT"]
    target = aux["target"]

    lse_row = np.zeros((B, N), np.float32)
    gsc_g = np.zeros((B, N), np.float32)
    m_part = np.empty((NCORES, 128, B), np.float32)   # col max partials
    s_part = np.empty((NCORES, 128, B), np.float32)
    for c in range(NCORES):
        o_row = results[c]["out_row"]                 # [3, 128, NRT]
        o_col = results[c]["out_col"]                 # [2, 128, B]
        tsel = aux["rows_of_core"][c]
        s_idx = np.arange(len(tsel))
        p, rt = s_idx % 128, s_idx // 128
        negm1 = o_row[0, p, rt]
        s1 = o_row[1, p, rt]
        gsc = o_row[2, p, rt]
        with np.errstate(divide="ignore"):
            lse = (-negm1 + np.log(s1)).astype(np.float32)
        bidx = np.array([b for b, t in tsel])
        tidx = np.array([t for b, t in tsel])
        ok = tidx < np.array([Ls[b] for b in bidx])   # ignore padding rows
        lse_row[bidx[ok], tidx[ok]] = lse[ok]
        gsc_g[bidx[ok], tidx[ok]] = gsc[ok]
        m_part[c] = -o_col[0]
        s_part[c] = o_col[1]

    M = m_part.max(axis=0)                            # [128, B]
    with np.errstate(invalid="ignore"):
        sc = (s_part * np.exp(m_part - M[None])).sum(axis=0)
    with np.errstate(divide="ignore"):
        lse_col = (M + np.log(sc)).T.astype(np.float32)  # [B, j]

    bt0 = np.float32(aux["bt"][0])
    lse_row = (lse_row + bt0).astype(np.float32)
    lse_col = (lse_col + bt0).astype(np.float32)

    bi = np.arange(B)[:, None]
    ti = np.arange(N)[None, :]
    g_bt = (gsc_g + bt0).astype(np.float32)
    row_m_at = aux["row_m"][bi, ti, target]
    col_m_at = aux["col_m"][bi, ti, target]
    e_row_at = np.where(row_m_at == 0, g_bt, NEG).astype(np.float32)
    e_col_at = np.where(col_m_at == 0, g_bt, NEG).astype(np.float32)
    lse_col_at = lse_col[bi, target].astype(np.float32)

    validt = aux["validj"]
    nll = np.where(validt, lse_row - e_row_at, np.float32(0)).astype(np.float32)
    nll2 = np.where(validt, lse_col_at - e_col_at, np.float32(0)).astype(np.float32)

    lens = aux["tgt_len"].astype(np.float32)
    d1 = (lens + np.float32(1e-20) - np.float32(1.0)).astype(np.float32)
    row_loss = np.float32(np.mean((nll.sum(axis=1) / d1).astype(np.float32)))
    col_loss = np.float32(np.mean((nll2.sum(axis=1) / (lens * d1)).astype(np.float32)))
    return np.asarray(row_loss + col_loss, dtype=np.float32)


def kernel(dec_outputs, sen_vec, Wq, bq, Wk, bk, wt, bt, target, tgt_len):
    in_maps, aux = host_prep_v2(
        dec_outputs, sen_vec, Wq, bq, Wk, bk, wt, bt, target, tgt_len
    )
    nc = _get_program_v2(aux["plan"])
    res = run_bass_kernel_spmd(nc, in_maps, core_ids=list(range(NCORES)))
    return host_combine_v2(res.results, aux)
